# revision 1
# baseline (speedup 1.0000x reference)
"""Trainium2 Bass kernel for nn_ConditionedConvolution2D.

Reference computation:
    A  = P @ dense_w                      # [B, 3*3*C*C_OUT] per-sample conv kernels
    Wk = A.reshape(B, 3, 3, C, C_OUT)
    Y[b] = conv2d(X[b], Wk[b])            # SAME padding, stride 1, NHWC

Strategy (pure data parallel, 4 samples per core on 8 cores):
  - Host pre-lays X as a bf16 "shifted triple" X_trip[b, dw*32+ci, hp, wp] =
    X_padded[b, ci, hp, wp+dw] so the device can read, for every padded row hp,
    a ready-made im2col stationary lhsT [96=(dw,ci), 128=w] with a single AP.
  - Device computes the hypernetwork (per-sample kernels) with 96 small
    matmuls from a host-permuted dense_w so the weights land directly in
    [(dw,ci), (dh_rev,co)] streaming layout, then casts to bf16.
  - Conv: PSUM accumulators are full banks holding 16 output rows
    ([128 w, 16*32 (row,co)]).  For each padded row hp, a single matmul with
    moving operand [96, 96] = [Wk(dh=2) | Wk(dh=1) | Wk(dh=0)] writes the
    three consecutive row-chunks r = hp-2, hp-1, hp of the bank in one shot
    (per-element has_written gives accumulate-or-init per chunk).  Matmuls
    split only at bank boundaries / sample edges -> ~1.1 matmuls per row.
  - Completed 16-row banks are copied PSUM->SBUF with a cast to bf16
    (alternating DVE/ACT) and DMA'd to a [b, w, h*co] DRAM layout
    (contiguous 1KB runs); the host transposes back to NHWC and upcasts.
"""

import os
import sys

sys.path.insert(0, "/opt/trn_rl_repo")

import numpy as np
import ml_dtypes

import concourse.bacc as bacc
import concourse.mybir as mybir
import concourse.tile as tile
from concourse.bass_utils import run_bass_kernel_spmd

B, H, W, C = 32, 128, 128, 32
P_DIM = 128
KH = KW = 3
C_OUT = 32
N_CORES = 8
BPC = B // N_CORES          # samples per core
H2 = H + 2                  # padded rows
W2 = W + 4                  # padded row pitch (2 pad cols + 2 alignment)
QK = KW * C                 # 96 contraction size (dw, ci)
G = KH * C_OUT              # 96 weight-stream columns per sample (dh_rev, co)
RPT = 16                    # output rows per PSUM tile (one full bank)

_NC_CACHE = {}


def _build_nc():
    f32 = mybir.dt.float32
    bf16 = mybir.dt.bfloat16
    nc = bacc.Bacc("TRN2", target_bir_lowering=False, debug=False,
                   num_devices=N_CORES)
    x_trip = nc.dram_tensor("x_trip", [BPC, QK, H2 * W2], bf16,
                            kind="ExternalInput")
    p_t = nc.dram_tensor("p_t", [P_DIM, BPC], bf16, kind="ExternalInput")
    dw_t = nc.dram_tensor("dw_t", [P_DIM, KH * KW * C * C_OUT], bf16,
                          kind="ExternalInput")
    y = nc.dram_tensor("y", [BPC, W, H * C_OUT], bf16, kind="ExternalOutput")

    with tile.TileContext(nc) as tc:
        with tc.tile_pool(name="const", bufs=1) as cpool, \
             tc.tile_pool(name="wsb", bufs=1) as wsb_pool, \
             tc.tile_pool(name="slab", bufs=2) as slab_pool, \
             tc.tile_pool(name="osb", bufs=4) as osb_pool:

            # ---- Phase 0: hypernetwork  Wk = P @ dense_w (permuted) ----
            p_sb = cpool.tile([P_DIM, BPC], bf16, name="p_sb", tag="p_sb")
            nc.sync.dma_start(out=p_sb[:], in_=p_t[:])
            dwsb = cpool.tile([P_DIM, KH * KW * C * C_OUT], bf16,
                              name="dwsb", tag="dwsb")
            nc.sync.dma_start(out=dwsb[:], in_=dw_t[:])

            # w_sb[q=(dw,ci), b*G + (2-dh)*C_OUT + co] (bf16 stream operand)
            w_sb = wsb_pool.tile([QK, BPC * G], bf16, name="w_sb", tag="w_sb")

            with tc.tile_pool(name="wps", bufs=2, space="PSUM") as wps_pool:
                for half in range(2):
                    wps = wps_pool.tile([QK, 48 * BPC], f32, name="wps",
                                        tag="wps")
                    for gg in range(48):
                        g = half * 48 + gg      # g = dh_rev*C_OUT + co
                        nc.tensor.matmul(
                            out=wps[:, gg * BPC:(gg + 1) * BPC],
                            lhsT=dwsb[:, g * QK:(g + 1) * QK],
                            rhs=p_sb[:],
                            start=True, stop=True,
                        )
                    # permute (g, b) -> (b, g) while casting f32 -> bf16
                    src = wps[:].rearrange("p (g b) -> p g b", b=BPC)
                    dst = w_sb[:].rearrange("p (b g) -> p g b", g=G)[
                        :, half * 48:(half + 1) * 48, :]
                    nc.vector.tensor_copy(out=dst, in_=src)

            # ---- Phase 1: per-sample conv ----
            with tc.tile_pool(name="acc", bufs=3, space="PSUM") as acc_pool:
                for b in range(BPC):
                    # split the slab load for finer DMA/compute overlap
                    slab = slab_pool.tile([QK, H2 * W2], bf16, name="slab",
                                          tag="slab")
                    HSPLIT = 65
                    nc.sync.dma_start(out=slab[:, :HSPLIT * W2],
                                      in_=x_trip[b][:, :HSPLIT * W2])
                    nc.sync.dma_start(out=slab[:, HSPLIT * W2:],
                                      in_=x_trip[b][:, HSPLIT * W2:])

                    tiles = {}      # t -> psum AP [W, RPT*C_OUT]
                    for hp in range(H2):
                        lhsT = slab[:, hp * W2: hp * W2 + W]
                        # output rows touched by this X row, oldest first
                        rows = [r for r in (hp - 2, hp - 1, hp)
                                if 0 <= r < H]
                        # group into runs within one PSUM tile
                        groups = []
                        for r in rows:
                            t = r // RPT
                            if groups and groups[-1][0] == t:
                                groups[-1][1].append(r)
                            else:
                                groups.append((t, [r]))
                        for t, rs in groups:
                            if t not in tiles:
                                tiles[t] = acc_pool.tile(
                                    [W, RPT * C_OUT], f32, name="acc",
                                    tag="acc")
                            r_lo, r_hi = rs[0], rs[-1]
                            c_lo = r_lo % RPT
                            # dh for row r is hp-r; col block index is 2-dh
                            w_lo = 2 - (hp - r_lo)
                            nc.tensor.matmul(
                                out=tiles[t][:, c_lo * C_OUT:
                                             (c_lo + len(rs)) * C_OUT],
                                lhsT=lhsT,
                                rhs=w_sb[:, b * G + w_lo * C_OUT:
                                         b * G + (w_lo + len(rs)) * C_OUT],
                                start=(r_lo % RPT == 0 and hp - r_lo == 0),
                                stop=(r_hi % RPT == RPT - 1
                                      and hp - r_hi == 2),
                                skip_group_check=True,
                            )
                        # tile t complete once row (t+1)*RPT-1 got its dh=2
                        t_done = None
                        if hp >= 2 and (hp - 2) % RPT == RPT - 1:
                            t_done = (hp - 2) // RPT
                        if t_done is not None:
                            osb = osb_pool.tile([W, RPT * C_OUT], bf16,
                                                name="osb", tag="osb")
                            src = tiles.pop(t_done)
                            if t_done % 2 == 0:
                                nc.vector.tensor_copy(out=osb[:], in_=src[:])
                            else:
                                nc.scalar.copy(out=osb[:], in_=src[:])
                            nc.sync.dma_start(
                                out=y[b][:, t_done * RPT * C_OUT:
                                         (t_done + 1) * RPT * C_OUT],
                                in_=osb[:],
                            )
    nc.finalize()
    return nc


def _get_nc():
    if "nc" not in _NC_CACHE:
        _NC_CACHE["nc"] = _build_nc()
    return _NC_CACHE["nc"]


def _prep_inputs(X, P, dense_w):
    bf16 = ml_dtypes.bfloat16
    Xb = np.ascontiguousarray(X.transpose(0, 3, 1, 2)).astype(bf16)  # [B,C,H,W]
    X_trip = np.zeros((B, QK, H2, W2), dtype=bf16)
    for dw in range(KW):
        lo = max(0, 1 - dw)          # first valid wp
        hi = W - dw                  # last valid wp (inclusive)
        src_lo = lo + dw - 1
        X_trip[:, dw * C:(dw + 1) * C, 1:H + 1, lo:hi + 1] = \
            Xb[:, :, :, src_lo:W]
    X_trip = X_trip.reshape(B, QK, H2 * W2)

    # dense_w columns j = ((dh*3+dw)*C+ci)*C_OUT+co -> (2-dh, co, dw, ci)
    dwp = np.ascontiguousarray(
        dense_w.reshape(P_DIM, KH, KW, C, C_OUT)[:, ::-1]
        .transpose(0, 1, 4, 2, 3)
        .reshape(P_DIM, -1)
    ).astype(bf16)

    in_maps = []
    for c in range(N_CORES):
        sl = slice(c * BPC, (c + 1) * BPC)
        in_maps.append({
            "x_trip": np.ascontiguousarray(X_trip[sl]),
            "p_t": np.ascontiguousarray(P[sl].T).astype(bf16),
            "dw_t": dwp,
        })
    return in_maps


def _run(X, P, dense_w, **spmd_kwargs):
    nc = _get_nc()
    in_maps = _prep_inputs(X, P, dense_w)
    res = run_bass_kernel_spmd(nc, in_maps, core_ids=list(range(N_CORES)),
                               **spmd_kwargs)
    outs = []
    for c in range(N_CORES):
        yv = res.results[c]["y"].astype(np.float32)
        yv = yv.reshape(BPC, W, H, C_OUT)
        outs.append(yv.transpose(0, 2, 1, 3))        # -> [b, h, w, co]
    Y = np.ascontiguousarray(np.concatenate(outs, axis=0), dtype=np.float32)
    return Y, res


def kernel(X, P, dense_w):
    Y, _ = _run(np.asarray(X), np.asarray(P), np.asarray(dense_w))
    return Y

